# revision 7
# baseline (speedup 1.0000x reference)
"""Fused BatchNorm1d(train) + block-diagonal GEMM + tanh + residual for TRN2.

  out = tanh(batchnorm(x) @ block_diag(W) + bias) + x,  x: [16384, 4096] fp32

Sharding: expert-style along features. Each of the 8 cores owns 512
features = 4 independent 128x128 blocks, and the full batch, so batch
stats need no collective.

Math: fold normalization into the weights. With s = gamma*rsqrt(var+eps),
t = beta - mean*s:
  y_p = xn_p @ W_p = x_p @ (s_p * W_p) + (t_p @ W_p)
so pass 2 is a plain GEMM with W'_p = s_p*W_p plus a per-output-feature
constant bias'' = bias + t@W, then tanh, then +x.

Pipeline per core (128 row-tiles of [128 batch, 512 feat]):
  Pass 1: DMA x in; cast to bf16 (ACT); one [128,129] matmul per block
          accumulates Gram (sum x^2 on diag) + batch sums in PSUM.
          Optionally PE-transposes some tiles (fp32, exact) and parks
          xT in SBUF for pass 2.
  Finalize: diag/sums -> mean/var -> s, t; scale W on ACT; build bias''
          and split into 3 bf16 rows for a K=3 PSUM bias-broadcast matmul.
  Pass 2: per row-tile: PE-transpose x blocks (fp32) unless parked;
          bias-preload matmul + 4 fp32 GEMMs into one PSUM bank; ACT
          tanh (PSUM->SBUF); DVE residual add; DMA out.
"""

import os
import sys

import numpy as np

for _p in ("/opt/trn_rl_repo", "/root/.axon_site/_ro/trn_rl_repo"):
    if _p not in sys.path and os.path.isdir(_p):
        sys.path.append(_p)

import ml_dtypes  # noqa: E402
import concourse.bass as bass  # noqa: E402
import concourse.tile as tile  # noqa: E402
from concourse import bacc, mybir  # noqa: E402
from concourse.bass_utils import run_bass_kernel_spmd  # noqa: E402

B = 16384          # batch
F = 4096           # features
NPART = 32         # independent blocks
D = 128            # block size
NCORES = 8
FS = F // NCORES   # features per core = 512
NBLK = FS // D     # blocks per core = 4
NT = B // 128      # row-tiles per core = 128
EPS = 1e-5

# Tunables (env-overridable for experiments)
T_RES = int(os.environ.get("KRN_T", "0"))    # xT-resident row-tiles
X_RES = int(os.environ.get("KRN_X", "0"))    # x-resident row-tiles
S2 = int(os.environ.get("KRN_S2", "2"))      # pass-2 super-tile
S1 = int(os.environ.get("KRN_S1", "4"))      # pass-1 super-tile
STATS_FP32 = os.environ.get("KRN_STATS_FP32", "0") == "1"

_CACHE: dict = {}


def _emit_body(nc, tc, ctx, pools, consts, x_d, out_d, it):
    """One full iteration: stats pass + finalize + apply pass, x_d -> out_d."""
    dt = mybir.dt
    (singles, p1_pool, bf_pool, stats_ps, xt_ps, y_ps, xt_work, p2_pool,
     o_pool, fin) = pools
    (ident, ones3, w_orig_f, bias_f, gcol_f, btcol_f) = consts

    def dram_rows(ap, t0, n):
        return ap[t0 * 128:(t0 + n) * 128, :].rearrange("(a p) f -> p a f", p=128)

    if T_RES:
        xt_res = singles.tile([128, T_RES * FS], dt.float32,
                              tag="xt_res", name="xt_res")
    if X_RES:
        x_res = singles.tile([128, X_RES * FS], dt.float32,
                             tag="x_res", name="x_res")

    def xt_res_slice(t):
        return xt_res[:, (t - X_RES) * FS:(t - X_RES + 1) * FS]

    # ---------------- Pass 1: stats (+ optional transposes) -------------
    sdt = dt.float32 if STATS_FP32 else dt.bfloat16
    gram = [stats_ps.tile([D, D + 1], dt.float32, tag=f"gram{p}",
                          name=f"gram{p}_{it}") for p in range(NBLK)]

    for st in range(NT // S1):
        t0 = st * S1
        if t0 < X_RES and t0 + S1 <= X_RES:
            x_src_sup = x_res[:, t0 * FS:(t0 + S1) * FS].rearrange(
                "p (a f) -> p a f", a=S1)
        else:
            x_src_sup = p1_pool.tile([128, S1, FS], dt.float32, tag="x1",
                                     name=f"x1_{it}_{st}")
        nc.sync.dma_start(out=x_src_sup, in_=dram_rows(x_d, t0, S1))

        for k in range(S1):
            t = t0 + k
            x_t = x_src_sup[:, k, :]
            xb = bf_pool.tile([128, NBLK, D + 1], sdt, tag="xb",
                              name=f"xb_{it}_{t}")
            nc.scalar.copy(
                out=xb[:, :, 0:D],
                in_=x_t.rearrange("p (blk d) -> p blk d", blk=NBLK))
            nc.vector.memset(xb[:, :, D:D + 1], 1.0)
            for p in range(NBLK):
                nc.tensor.matmul(
                    gram[p], lhsT=xb[:, p, 0:D], rhs=xb[:, p, :],
                    start=(t == 0), stop=(t == NT - 1))
            if X_RES <= t < X_RES + T_RES:
                xt_p = xt_ps.tile([128, FS], dt.float32, tag="xtp",
                                  name=f"xtp1_{it}_{t}")
                for p in range(NBLK):
                    nc.tensor.transpose(
                        xt_p[:, p * D:(p + 1) * D],
                        x_t[:, p * D:(p + 1) * D], ident)
                nc.vector.tensor_copy(out=xt_res_slice(t), in_=xt_p)

    # ---------------- Finalize: stats -> scaled weights ------------------
    def ftile(nm, shape=(D, NBLK)):
        return fin.tile(list(shape), dt.float32, tag=nm, name=f"{nm}_{it}")

    sums = ftile("sums")
    ssq = ftile("ssq")
    for p in range(NBLK):
        nc.vector.tensor_copy(out=sums[:, p:p + 1], in_=gram[p][:, D:D + 1])
        dtmp = fin.tile([D, D], dt.float32, tag=f"dtmp{p}", name=f"dtmp{p}_{it}")
        nc.vector.tensor_mul(dtmp, gram[p][:, 0:D], ident)
        nc.vector.tensor_reduce(
            out=ssq[:, p:p + 1], in_=dtmp, axis=mybir.AxisListType.X,
            op=mybir.AluOpType.add)

    mean = ftile("mean")
    nc.scalar.mul(mean, sums, 1.0 / B)
    var = ftile("var")
    nc.scalar.mul(var, ssq, 1.0 / B)
    m2 = ftile("m2")
    nc.vector.tensor_mul(m2, mean, mean)
    nc.vector.tensor_sub(var, var, m2)
    veps = ftile("veps")
    nc.vector.tensor_scalar_add(veps, var, EPS)
    std = ftile("std")
    nc.scalar.sqrt(std, veps)
    rstd = ftile("rstd")
    nc.vector.reciprocal(rstd, std)
    nt1 = ftile("nt1")
    nc.vector.tensor_mul(nt1, veps, rstd)
    nc.vector.tensor_mul(nt1, nt1, rstd)          # v*r^2
    nc.vector.tensor_scalar(nt1, nt1, -0.5, 1.5,
                            mybir.AluOpType.mult, mybir.AluOpType.add)
    nc.vector.tensor_mul(rstd, rstd, nt1)         # r *= 1.5 - 0.5*v*r^2

    s_c = ftile("s_c")
    nc.vector.tensor_mul(s_c, gcol_f, rstd)
    t_c = ftile("t_c")
    nc.vector.tensor_mul(t_c, mean, s_c)
    nc.vector.tensor_sub(t_c, btcol_f, t_c)       # t = beta - mean*s

    w_s = singles.tile([D, NBLK, D], dt.float32, tag="w_s", name=f"w_s_{it}")
    c_ps = stats_ps.tile([1, FS], dt.float32, tag="gram0", name=f"c_ps_{it}")
    for p in range(NBLK):
        nc.scalar.activation(
            out=w_s[:, p, :], in_=w_orig_f[:, p, :],
            func=mybir.ActivationFunctionType.Copy, scale=s_c[:, p:p + 1])
        nc.tensor.matmul(c_ps[:, p * D:(p + 1) * D], lhsT=t_c[:, p:p + 1],
                         rhs=w_orig_f[:, p, :], start=True, stop=True)
    bias2 = ftile("bias2", (1, FS))
    nc.vector.tensor_copy(out=bias2, in_=c_ps)
    nc.vector.tensor_add(bias2, bias2, bias_f)
    # split bias'' into 3 bf16 components (sum reconstructs ~fp32 exactly)
    bias_hl = singles.tile([3, FS], dt.bfloat16, tag="bias_hl",
                           name=f"bias_hl_{it}")
    rem = ftile("rem", (1, FS))
    rem2 = ftile("rem2", (1, FS))
    bc0 = fin.tile([1, FS], dt.bfloat16, tag="bc0", name=f"bc0_{it}")
    bc1 = fin.tile([1, FS], dt.bfloat16, tag="bc1", name=f"bc1_{it}")
    bc2 = fin.tile([1, FS], dt.bfloat16, tag="bc2", name=f"bc2_{it}")
    nc.vector.tensor_copy(out=bc0, in_=bias2)
    nc.vector.tensor_sub(rem, bias2, bc0)
    nc.vector.tensor_copy(out=bc1, in_=rem)
    nc.vector.tensor_sub(rem2, rem, bc1)
    nc.vector.tensor_copy(out=bc2, in_=rem2)
    for _i, _bc in enumerate([bc0, bc1, bc2]):
        nc.gpsimd.dma_start(out=bias_hl[_i:_i + 1, :], in_=_bc)

    # ---------------- Pass 2: GEMM + tanh + residual ---------------------
    for st in range(NT // S2):
        t0 = st * S2
        if t0 + S2 <= X_RES:
            x_sup = x_res[:, t0 * FS:(t0 + S2) * FS].rearrange(
                "p (a f) -> p a f", a=S2)
        else:
            x_sup = p2_pool.tile([128, S2, FS], dt.float32, tag="x2",
                                 name=f"x2_{it}_{st}")
            nc.sync.dma_start(out=x_sup, in_=dram_rows(x_d, t0, S2))
        o_sup = o_pool.tile([128, S2, FS], dt.float32, tag="o2",
                            name=f"o2_{it}_{st}")

        for k in range(S2):
            t = t0 + k
            x_t = x_sup[:, k, :]
            if X_RES <= t < X_RES + T_RES:
                xt = xt_res_slice(t)
            else:
                xt_p = xt_ps.tile([128, FS], dt.float32, tag="xtp",
                                  name=f"xtp2_{it}_{t}")
                for p in range(NBLK):
                    nc.tensor.transpose(
                        xt_p[:, p * D:(p + 1) * D],
                        x_t[:, p * D:(p + 1) * D], ident)
                xt = xt_work.tile([128, FS], dt.float32, tag="xtw",
                                  name=f"xtw_{it}_{t}")
                nc.vector.tensor_copy(out=xt, in_=xt_p)

            y = y_ps.tile([128, FS], dt.float32, tag="y", name=f"y_{it}_{t}")
            nc.tensor.matmul(y, lhsT=ones3, rhs=bias_hl, start=True, stop=False)
            for p in range(NBLK):
                nc.tensor.matmul(
                    y[:, p * D:(p + 1) * D], lhsT=xt[:, p * D:(p + 1) * D],
                    rhs=w_s[:, p, :], start=False, stop=(p == NBLK - 1))
            o_t = o_sup[:, k, :]
            nc.scalar.activation(out=o_t, in_=y,
                                 func=mybir.ActivationFunctionType.Tanh)
            nc.vector.tensor_add(o_t, o_t, x_t)

        nc.sync.dma_start(out=dram_rows(out_d, t0, S2), in_=o_sup)


def build(chain=1):
    """Build + compile the SPMD program. chain>1 loops the body through
    internal DRAM buffers (for slope timing)."""
    nc = bacc.Bacc("TRN2", target_bir_lowering=False, debug=False)
    dt = mybir.dt
    x_d = nc.dram_tensor("x", [B, FS], dt.float32, kind="ExternalInput").ap()
    w_d = nc.dram_tensor("w", [NBLK, D, D], dt.float32, kind="ExternalInput").ap()
    bias_d = nc.dram_tensor("b", [FS], dt.float32, kind="ExternalInput").ap()
    gamma_d = nc.dram_tensor("g", [FS], dt.float32, kind="ExternalInput").ap()
    beta_d = nc.dram_tensor("bt", [FS], dt.float32, kind="ExternalInput").ap()
    id_d = nc.dram_tensor("ident", [D, D], dt.float32, kind="ExternalInput").ap()
    ones3_d = nc.dram_tensor("ones3", [3, D], dt.bfloat16, kind="ExternalInput").ap()
    out_d = nc.dram_tensor("out", [B, FS], dt.float32, kind="ExternalOutput").ap()
    # unused input whose shape depends on chain: breaks HLO/NEFF cache
    # collisions between chain variants (all real in/outs have fixed shapes)
    nc.dram_tensor("salt", [chain, 1], dt.float32, kind="ExternalInput")
    scratch = [nc.dram_tensor(f"scr{i}", [B, FS], dt.float32).ap()
               for i in range(min(chain - 1, 2))]

    import contextlib
    with tile.TileContext(nc) as tc, contextlib.ExitStack() as ctx:
        singles = ctx.enter_context(tc.tile_pool(name="singles", bufs=1))
        p1_pool = ctx.enter_context(tc.tile_pool(name="p1", bufs=3))
        bf_pool = ctx.enter_context(tc.tile_pool(name="bf", bufs=3))
        stats_ps = ctx.enter_context(tc.tile_pool(name="stats_ps", bufs=1, space="PSUM"))
        xt_ps = ctx.enter_context(tc.tile_pool(name="xt_ps", bufs=2, space="PSUM"))
        y_ps = ctx.enter_context(tc.tile_pool(name="y_ps", bufs=2, space="PSUM"))
        xt_work = ctx.enter_context(tc.tile_pool(name="xt_work", bufs=3))
        p2_pool = ctx.enter_context(tc.tile_pool(name="p2", bufs=3))
        o_pool = ctx.enter_context(tc.tile_pool(name="o", bufs=3))
        fin = ctx.enter_context(tc.tile_pool(name="fin", bufs=1))
        pools = (singles, p1_pool, bf_pool, stats_ps, xt_ps, y_ps, xt_work,
                 p2_pool, o_pool, fin)

        ident = singles.tile([D, D], dt.float32, tag="ident", name="ident")
        nc.sync.dma_start(out=ident, in_=id_d)
        ones3 = singles.tile([3, D], dt.bfloat16, tag="ones3", name="ones3")
        nc.sync.dma_start(out=ones3, in_=ones3_d)
        w_orig = singles.tile([D, NBLK, D], dt.float32, tag="w_orig", name="w_orig")
        nc.sync.dma_start(out=w_orig, in_=w_d.rearrange("blk i j -> i blk j"))
        brow = singles.tile([1, FS], dt.float32, tag="brow", name="brow")
        nc.sync.dma_start(out=brow, in_=bias_d[None, :])
        gcol = singles.tile([D, NBLK], dt.float32, tag="gcol", name="gcol")
        nc.gpsimd.dma_start(out=gcol, in_=gamma_d.rearrange("(p i) -> i p", p=NBLK))
        btcol = singles.tile([D, NBLK], dt.float32, tag="btcol", name="btcol")
        nc.gpsimd.dma_start(out=btcol, in_=beta_d.rearrange("(p i) -> i p", p=NBLK))
        consts = (ident, ones3, w_orig, brow, gcol, btcol)

        for it in range(chain):
            src = x_d if it == 0 else scratch[(it - 1) % 2]
            dst = out_d if it == chain - 1 else scratch[it % 2]
            _emit_body(nc, tc, ctx, pools, consts, src, dst, it)

    nc.compile()
    return nc


def _get_nc():
    key = (T_RES, X_RES, S2, S1, STATS_FP32, 1)
    if key not in _CACHE:
        _CACHE[key] = build(1)
    return _CACHE[key]


# back-compat alias used by test.py
def _build():
    return _get_nc()


def make_in_maps(x, weights, bias, gamma, beta, chain=1):
    ident = np.eye(D, dtype=np.float32)
    ones3 = np.ones((3, D), dtype=ml_dtypes.bfloat16)
    in_maps = []
    for c in range(NCORES):
        f0 = c * FS
        in_maps.append({
            "x": np.ascontiguousarray(x[:, f0:f0 + FS]),
            "w": np.ascontiguousarray(weights[c * NBLK:(c + 1) * NBLK]),
            "b": np.ascontiguousarray(bias[f0:f0 + FS]),
            "g": np.ascontiguousarray(gamma[f0:f0 + FS]),
            "bt": np.ascontiguousarray(beta[f0:f0 + FS]),
            "ident": ident,
            "ones3": ones3,
            "salt": np.zeros((chain, 1), np.float32),
        })
    return in_maps


def kernel(**inputs) -> np.ndarray:
    x = np.ascontiguousarray(inputs["x"], dtype=np.float32)
    weights = np.ascontiguousarray(inputs["weights"], dtype=np.float32)
    bias = np.ascontiguousarray(inputs["bias"], dtype=np.float32)
    gamma = np.ascontiguousarray(inputs["gamma"], dtype=np.float32)
    beta = np.ascontiguousarray(inputs["beta"], dtype=np.float32)

    nc = _get_nc()
    in_maps = make_in_maps(x, weights, bias, gamma, beta)
    res = run_bass_kernel_spmd(nc, in_maps, list(range(NCORES)))
    out = np.concatenate([res.results[c]["out"] for c in range(NCORES)], axis=1)
    return out.astype(np.float32)


if __name__ == "__main__":
    rng = np.random.default_rng(0)
    ins = {
        "x": rng.standard_normal((B, F), dtype=np.float32),
        "weights": (rng.standard_normal((NPART, D, D), dtype=np.float32)
                    / np.sqrt(D)).astype(np.float32),
        "bias": rng.standard_normal(F, dtype=np.float32) * 0.1,
        "gamma": np.ones(F, dtype=np.float32),
        "beta": np.zeros(F, dtype=np.float32),
    }
    out = kernel(**ins)
    xn = (ins["x"] - ins["x"].mean(0)) / np.sqrt(ins["x"].var(0) + EPS)
    xn = xn * ins["gamma"] + ins["beta"]
    y = np.einsum("bpi,pij->bpj", xn.reshape(B, NPART, D),
                  ins["weights"]).reshape(B, F)
    ref = np.tanh(y + ins["bias"]) + ins["x"]
    err = np.abs(out - ref).max()
    print("abs err:", err, "rel:", err / np.abs(ref).max())
